# revision 1
# baseline (speedup 1.0000x reference)
"""Trainium2 Bass kernel for the LGP-instruction module (read -> op bank -> write).

Data-parallel over batch: core b computes x[b] (2048, 4096).
Device pipeline per core:
  phase 1: valuesT[C, T] = sum_vt rw_tile[vt].T @ xT_tile[vt]   (PSUM, 4 banks)
  phase 2: per T-chunk of 512:
     h_i = W_i.T @ valuesT  (PSUM) -> ACT f_i(h + b_i) -> DVE weighted-accumulate
     out[Tsub, V] = accT.T @ wwT  -> DVE copy -> DMA store
Host prep: read_w softmax, write_w*out_scale transpose, x[b].T layout.
Matmuls run as float32r (fp32 bits, full-rate PE streaming).
"""
import sys
import numpy as np

if '/opt/trn_rl_repo' not in sys.path:
    sys.path.insert(0, '/opt/trn_rl_repo')

B, T, V, C, NOPS = 8, 2048, 4096, 128, 8
NCORES = 8
NV = V // 128     # 32 v-tiles
NTC = T // 512    # 4 T-chunks

_CACHE = {}
LAST_RESULT = None


def _build(pre, post):
    from concourse import bass, bacc, tile, mybir
    f32, f32r = mybir.dt.float32, mybir.dt.float32r
    AF = mybir.ActivationFunctionType
    ts = bass.ts
    FUNCS = [AF.Identity, AF.Relu, AF.Gelu, AF.Square,
             AF.Identity, AF.Abs, AF.Tanh, AF.Sigmoid]

    nc = bacc.Bacc("TRN2", target_bir_lowering=False, debug=False,
                   num_devices=NCORES)
    xT = nc.dram_tensor("xT", [V, T], f32r, kind="ExternalInput")
    rw = nc.dram_tensor("rw", [V, C], f32r, kind="ExternalInput")
    wwT = nc.dram_tensor("wwT", [C, V], f32r, kind="ExternalInput")
    opw = nc.dram_tensor("opw", [NOPS, C, C], f32r, kind="ExternalInput")
    opb = nc.dram_tensor("opb", [C, NOPS], f32, kind="ExternalInput")
    out = nc.dram_tensor("out", [T, V], f32, kind="ExternalOutput")

    NBLK = 4          # xT load blocks per T-chunk
    VB = NV // NBLK   # 8 v-tiles per block

    # xT viewed as [p, vtile, t]
    xTr = xT.ap().rearrange("(vb p) t -> p vb t", p=128)

    with tile.TileContext(nc) as tc:
        with tc.tile_pool(name="const", bufs=1) as constp, \
             tc.tile_pool(name="xt", bufs=6) as xtp, \
             tc.tile_pool(name="vals_ps", bufs=2, space="PSUM") as vpsp, \
             tc.tile_pool(name="vals_sb", bufs=2) as vsbp, \
             tc.tile_pool(name="h_ps", bufs=3, space="PSUM") as hpsp, \
             tc.tile_pool(name="t_sb", bufs=3) as tp, \
             tc.tile_pool(name="acc", bufs=2) as accp, \
             tc.tile_pool(name="out_ps", bufs=3, space="PSUM") as opsp, \
             tc.tile_pool(name="out_sb", bufs=2) as osbp:

            rw_t = constp.tile([128, NV, C], f32r)
            nc.sync.dma_start(rw_t[:], rw.ap().rearrange("(vt p) c -> p vt c", p=128))
            wwT_t = constp.tile([C, V], f32r)
            nc.sync.dma_start(wwT_t[:], wwT.ap())
            opw_t = constp.tile([C, NOPS, C], f32r)
            nc.sync.dma_start(opw_t[:], opw.ap().rearrange("i p c -> p i c"))
            opb_t = constp.tile([C, NOPS], f32)
            nc.sync.dma_start(opb_t[:], opb.ap())

            for tcn in range(NTC):
                # read: accumulate over all V into one psum bank
                values = vpsp.tile([128, 512], f32)
                for blk in range(NBLK):
                    xt = xtp.tile([128, VB, 512], f32r)
                    nc.sync.dma_start(
                        xt[:], xTr[:, ts(blk, VB), ts(tcn, 512)])
                    for j in range(VB):
                        vt = blk * VB + j
                        nc.tensor.matmul(values[:], rw_t[:, vt, :], xt[:, j, :],
                                         start=(vt == 0), stop=(vt == NV - 1))
                vals = vsbp.tile([128, 512], f32r)
                nc.vector.tensor_copy(vals[:], values[:])

                # op bank
                acc = accp.tile([128, 512], f32r)
                for i in range(NOPS):
                    h = hpsp.tile([128, 512], f32)
                    nc.tensor.matmul(h[:], opw_t[:, i, :], vals[:],
                                     start=True, stop=True)
                    if i == 0:
                        nc.scalar.activation(acc[:], h[:], FUNCS[0],
                                             bias=opb_t[:, 0:1], scale=pre[0])
                    else:
                        t = tp.tile([128, 512], f32r)
                        nc.scalar.activation(t[:], h[:], FUNCS[i],
                                             bias=opb_t[:, i:i + 1], scale=pre[i])
                        nc.vector.scalar_tensor_tensor(
                            acc[:], t[:], post[i], acc[:],
                            op0=mybir.AluOpType.mult, op1=mybir.AluOpType.add)

                # write: out rows, stores on SWDGE so loads never queue behind them
                for sub in range(4):
                    osb = osbp.tile([128, V], f32)
                    for nn in range(8):
                        ops_ = opsp.tile([128, 512], f32)
                        nc.tensor.matmul(ops_[:], acc[:, ts(sub, 128)],
                                         wwT_t[:, ts(nn, 512)],
                                         start=True, stop=True)
                        idx = (tcn * 4 + sub) * 8 + nn
                        if idx % 9 < 2:   # ~2/9 of psum-drain copies go to ACT
                            nc.scalar.copy(osb[:, ts(nn, 512)], ops_[:])
                        else:
                            nc.vector.tensor_copy(osb[:, ts(nn, 512)], ops_[:])
                    nc.gpsimd.dma_start(out.ap()[ts(tcn * 4 + sub, 128), :], osb[:])
    nc.compile()
    return nc


def _softmax(x, axis):
    x = np.asarray(x, np.float32)
    m = x.max(axis=axis, keepdims=True)
    e = np.exp(x - m)
    return e / e.sum(axis=axis, keepdims=True)


def kernel(x, basis, read_coeffs, write_coeffs, op_logits, op_weights,
           op_biases, out_scale):
    global LAST_RESULT
    from concourse.bass_utils import run_bass_kernel_spmd

    x = np.asarray(x, np.float32)
    basis = np.asarray(basis, np.float32)
    read_coeffs = np.asarray(read_coeffs, np.float32)
    write_coeffs = np.asarray(write_coeffs, np.float32)
    op_logits = np.asarray(op_logits, np.float32)
    op_weights = np.asarray(op_weights, np.float32)
    op_biases = np.asarray(op_biases, np.float32)
    out_scale = np.float32(out_scale)

    read_w = _softmax(basis @ read_coeffs.T, axis=0)               # (V, C)
    wwT = np.ascontiguousarray((basis @ write_coeffs.T).T) * out_scale  # (C, V)
    w = _softmax(op_logits, axis=0).astype(np.float64)

    # fold the mixture weight into ACT scale/bias where the nonlinearity allows
    #   i: 0 ident, 1 relu, 2 gelu, 3 square, 4 neg, 5 abs, 6 tanh, 7 sigmoid
    pre = [w[0], w[1], 1.0, np.sqrt(w[3]), -w[4], w[5], 1.0, 1.0]
    post = [1.0, 1.0, w[2], 1.0, 1.0, 1.0, w[6], w[7]]
    pre = [float(v) for v in pre]
    post = [float(v) for v in post]

    key = tuple(pre) + tuple(post)
    if key not in _CACHE:
        _CACHE[key] = _build(pre, post)
    nc = _CACHE[key]

    opb = (op_biases.T * np.array(pre, np.float64)[None, :]).astype(np.float32)
    # gelu/tanh/sigmoid biases enter before the nonlinearity unscaled
    for i in (2, 6, 7):
        opb[:, i] = op_biases[i]

    shared = {
        "rw": read_w,
        "wwT": wwT.astype(np.float32),
        "opw": op_weights,
        "opb": np.ascontiguousarray(opb),
    }
    in_maps = []
    for b in range(B):
        m = dict(shared)
        m["xT"] = np.ascontiguousarray(x[b].T)
        in_maps.append(m)

    res = run_bass_kernel_spmd(nc, in_maps, core_ids=list(range(NCORES)))
    LAST_RESULT = res
    out = np.empty((B, T, V), np.float32)
    for b in range(B):
        out[b] = res.results[b]["out"]
    return out



# revision 3
# speedup vs baseline: 1.4739x; 1.4739x over previous
"""Trainium2 Bass kernel for the LGP-instruction module (read -> op bank -> write).

Data-parallel over batch: core b computes x[b] (2048, 4096).
All wire/SBUF tensors are bf16 (error gate 2e-2; bf16 end-to-end is ~3e-3).
Device pipeline per core, two T-chunks of 1024:
  phase 1: valuesT[C, TC] = sum_vt rw_tile[vt].T @ xT_tile[vt]   (PSUM, 2 banks)
  phase 2: per 512-half: h_i = W_i.T @ valuesT (PSUM) -> ACT f_i(h+b_i) -> DVE acc
  phase 3: out[128 rows, V] = accT.T @ wwT  -> drain (DVE/ACT/Pool) -> DMA store
Host prep: read_w softmax (packed [128, vt*C]), write_w*out_scale transpose,
x[b].T layout, all cast to bf16.
"""
import sys
import numpy as np

if '/opt/trn_rl_repo' not in sys.path:
    sys.path.insert(0, '/opt/trn_rl_repo')

B, T, V, C, NOPS = 8, 2048, 4096, 128, 8
NCORES = 8
NV = V // 128     # 32 v-tiles
TC = 1024         # T-chunk
NTC = T // TC     # 2 T-chunks

_CACHE = {}
LAST_RESULT = None


def _build(pre, post):
    from concourse import bass, bacc, tile, mybir
    f32, bf16 = mybir.dt.float32, mybir.dt.bfloat16
    AF = mybir.ActivationFunctionType
    ts = bass.ts
    FUNCS = [AF.Identity, AF.Relu, AF.Gelu, AF.Square,
             AF.Identity, AF.Abs, AF.Tanh, AF.Sigmoid]

    nc = bacc.Bacc("TRN2", target_bir_lowering=False, debug=False,
                   num_devices=NCORES)
    xT = nc.dram_tensor("xT", [V, T], bf16, kind="ExternalInput")
    rw = nc.dram_tensor("rw", [128, NV * C], bf16, kind="ExternalInput")
    wwT = nc.dram_tensor("wwT", [C, V], bf16, kind="ExternalInput")
    opw = nc.dram_tensor("opw", [C, NOPS * C], bf16, kind="ExternalInput")
    opb = nc.dram_tensor("opb", [C, NOPS], f32, kind="ExternalInput")
    out = nc.dram_tensor("out", [T, V], bf16, kind="ExternalOutput")

    NBLK = 4          # xT load blocks per T-chunk
    VB = NV // NBLK   # 8 v-tiles per block

    # xT viewed as [p, vtile, t]
    xTr = xT.ap().rearrange("(vb p) t -> p vb t", p=128)

    with tile.TileContext(nc) as tc:
        with tc.tile_pool(name="const", bufs=1) as constp, \
             tc.tile_pool(name="xt", bufs=5) as xtp, \
             tc.tile_pool(name="vals_ps", bufs=2, space="PSUM") as vpsp, \
             tc.tile_pool(name="vals_sb", bufs=2) as vsbp, \
             tc.tile_pool(name="h_ps", bufs=2, space="PSUM") as hpsp, \
             tc.tile_pool(name="t_sb", bufs=3) as tp, \
             tc.tile_pool(name="acc", bufs=2) as accp, \
             tc.tile_pool(name="out_ps", bufs=2, space="PSUM") as opsp, \
             tc.tile_pool(name="out_sb", bufs=2) as osbp:

            rw_t = constp.tile([128, NV, C], bf16)
            nc.sync.dma_start(rw_t[:], rw.ap().rearrange("p (vt c) -> p vt c", c=C))
            wwT_t = constp.tile([C, V], bf16)
            nc.sync.dma_start(wwT_t[:], wwT.ap())
            opw_t = constp.tile([C, NOPS, C], bf16)
            nc.sync.dma_start(opw_t[:], opw.ap().rearrange("p (i c) -> p i c", c=C))
            opb_t = constp.tile([C, NOPS], f32)
            nc.sync.dma_start(opb_t[:], opb.ap())

            for tcn in range(NTC):
                # read: accumulate over all V into a 2-bank psum tile
                values = vpsp.tile([128, TC], f32)
                for blk in range(NBLK):
                    xt = xtp.tile([128, VB, TC], bf16)
                    nc.sync.dma_start(
                        xt[:], xTr[:, ts(blk, VB), ts(tcn, TC)])
                    for j in range(VB):
                        vt = blk * VB + j
                        for hh in range(2):
                            nc.tensor.matmul(
                                values[:, ts(hh, 512)], rw_t[:, vt, :],
                                xt[:, j, ts(hh, 512)],
                                start=(vt == 0), stop=(vt == NV - 1))
                vals = vsbp.tile([128, TC], bf16)
                for hh in range(2):
                    nc.vector.tensor_copy(vals[:, ts(hh, 512)],
                                          values[:, ts(hh, 512)])

                # op bank (per 512-half)
                acc = accp.tile([128, TC], bf16)
                for i in range(NOPS):
                    for hh in range(2):
                        h = hpsp.tile([128, 512], f32)
                        nc.tensor.matmul(h[:], opw_t[:, i, :],
                                         vals[:, ts(hh, 512)],
                                         start=True, stop=True)
                        if i == 0:
                            nc.scalar.activation(acc[:, ts(hh, 512)], h[:],
                                                 FUNCS[0], bias=opb_t[:, 0:1],
                                                 scale=pre[0])
                        else:
                            t = tp.tile([128, 512], bf16)
                            nc.scalar.activation(t[:], h[:], FUNCS[i],
                                                 bias=opb_t[:, i:i + 1],
                                                 scale=pre[i])
                            nc.vector.scalar_tensor_tensor(
                                acc[:, ts(hh, 512)], t[:], post[i],
                                acc[:, ts(hh, 512)],
                                op0=mybir.AluOpType.mult,
                                op1=mybir.AluOpType.add)

                # write: out rows; stores on SWDGE so loads never queue behind
                for sub in range(TC // 128):
                    osb = osbp.tile([128, V], bf16)
                    for nn in range(8):
                        ops_ = opsp.tile([128, 512], f32)
                        nc.tensor.matmul(ops_[:], acc[:, ts(sub, 128)],
                                         wwT_t[:, ts(nn, 512)],
                                         start=True, stop=True)
                        idx = (tcn * (TC // 128) + sub) * 8 + nn
                        if idx % 3 == 0:
                            nc.scalar.copy(osb[:, ts(nn, 512)], ops_[:])
                        else:
                            nc.vector.tensor_copy(osb[:, ts(nn, 512)], ops_[:])
                    nc.gpsimd.dma_start(
                        out.ap()[ts(tcn * (TC // 128) + sub, 128), :], osb[:])
    nc.compile()
    return nc


def _softmax(x, axis):
    x = np.asarray(x, np.float32)
    m = x.max(axis=axis, keepdims=True)
    e = np.exp(x - m)
    return e / e.sum(axis=axis, keepdims=True)


def kernel(x, basis, read_coeffs, write_coeffs, op_logits, op_weights,
           op_biases, out_scale):
    global LAST_RESULT
    import ml_dtypes
    from concourse.bass_utils import run_bass_kernel_spmd
    bf16 = ml_dtypes.bfloat16

    x = np.asarray(x, np.float32)
    basis = np.asarray(basis, np.float32)
    read_coeffs = np.asarray(read_coeffs, np.float32)
    write_coeffs = np.asarray(write_coeffs, np.float32)
    op_logits = np.asarray(op_logits, np.float32)
    op_weights = np.asarray(op_weights, np.float32)
    op_biases = np.asarray(op_biases, np.float32)
    out_scale = np.float32(out_scale)

    read_w = _softmax(basis @ read_coeffs.T, axis=0)               # (V, C)
    # pack read_w to [p, vt*C] so the const load is contiguous 8KB lines
    rw_packed = np.ascontiguousarray(
        read_w.reshape(NV, 128, C).transpose(1, 0, 2).reshape(128, NV * C))
    wwT = np.ascontiguousarray((basis @ write_coeffs.T).T) * out_scale  # (C, V)
    w = _softmax(op_logits, axis=0).astype(np.float64)

    # fold the mixture weight into ACT scale/bias where the nonlinearity allows
    #   i: 0 ident, 1 relu, 2 gelu, 3 square, 4 neg, 5 abs, 6 tanh, 7 sigmoid
    pre = [w[0], w[1], 1.0, np.sqrt(w[3]), -w[4], w[5], 1.0, 1.0]
    post = [1.0, 1.0, w[2], 1.0, 1.0, 1.0, w[6], w[7]]
    pre = [float(v) for v in pre]
    post = [float(v) for v in post]

    key = tuple(pre) + tuple(post)
    if key not in _CACHE:
        _CACHE[key] = _build(pre, post)
    nc = _CACHE[key]

    opb = (op_biases.T * np.array(pre, np.float64)[None, :]).astype(np.float32)
    # gelu/tanh/sigmoid biases enter before the nonlinearity unscaled
    for i in (2, 6, 7):
        opb[:, i] = op_biases[i]

    # opw packed to [p, i*C]
    opw_packed = np.ascontiguousarray(
        op_weights.transpose(1, 0, 2).reshape(C, NOPS * C))

    shared = {
        "rw": rw_packed.astype(bf16),
        "wwT": wwT.astype(bf16),
        "opw": opw_packed.astype(bf16),
        "opb": np.ascontiguousarray(opb),
    }
    in_maps = []
    for b in range(B):
        m = dict(shared)
        m["xT"] = np.ascontiguousarray(x[b].T).astype(bf16)
        in_maps.append(m)

    res = run_bass_kernel_spmd(nc, in_maps, core_ids=list(range(NCORES)))
    LAST_RESULT = res
    out = np.empty((B, T, V), np.float32)
    for b in range(B):
        out[b] = res.results[b]["out"].astype(np.float32)
    return out


# revision 11
# speedup vs baseline: 1.5143x; 1.0274x over previous
"""Trainium2 Bass kernel for the LGP-instruction module (read -> op bank -> write).

Data-parallel over batch: core b computes x[b] (2048, 4096).
Precision plan (gate 2e-2, measured ~3.3e-3): x and read_w in fp8-e4m3
(read_w scaled x256, dequant folded into op weights), everything else bf16.
Read phase uses DoubleRow fp8 matmuls (K=256 per instruction).

Per core: x stays SBUF-resident; both T-chunks' values accumulate block-by-
block as x streams in, then per chunk: op bank (ACT+DVE nonlins, GpSimd
mixture tree) and the write projection acc.T @ wwT with drains split
DVE/ACT and stores on SWDGE.
"""
import sys
import numpy as np

if '/opt/trn_rl_repo' not in sys.path:
    sys.path.insert(0, '/opt/trn_rl_repo')

B, T, V, C, NOPS = 8, 2048, 4096, 128, 8
NCORES = 8
NV = V // 128     # 32 v-tiles
NP = NV // 2      # 16 v-tile pairs (DoubleRow)
TC = 1024         # T-chunk
NTC = T // TC     # 2 T-chunks
RW_SCALE = 256.0

_CACHE = {}
LAST_RESULT = None


def _build(pre, post):
    from concourse import bass, bacc, tile, mybir
    f32, bf16, f8 = mybir.dt.float32, mybir.dt.bfloat16, mybir.dt.float8e4
    AF = mybir.ActivationFunctionType
    DR = mybir.MatmulPerfMode.DoubleRow
    ts = bass.ts
    alu = mybir.AluOpType

    nc = bacc.Bacc("TRN2", target_bir_lowering=False, debug=False,
                   num_devices=NCORES)
    xT = nc.dram_tensor("xT", [V, T], f8, kind="ExternalInput")
    rw = nc.dram_tensor("rw", [128, NV * C], f8, kind="ExternalInput")
    wwT = nc.dram_tensor("wwT", [C, V], bf16, kind="ExternalInput")
    opw = nc.dram_tensor("opw", [C, NOPS * C], bf16, kind="ExternalInput")
    opb = nc.dram_tensor("opb", [C, NOPS], f32, kind="ExternalInput")
    out = nc.dram_tensor("out", [T, V], bf16, kind="ExternalOutput")

    NBLK = 4          # xT load blocks (full T each)
    VB = NV // NBLK   # 8 v-tiles per block

    xTr = xT.ap().rearrange("(vb p) t -> p vb t", p=128)

    with tile.TileContext(nc) as tc:
        with tc.tile_pool(name="const", bufs=1) as constp, \
             tc.tile_pool(name="xt", bufs=1) as xtp, \
             tc.tile_pool(name="vals_ps", bufs=1, space="PSUM") as vpsp, \
             tc.tile_pool(name="vals_sb", bufs=2) as vsbp, \
             tc.tile_pool(name="h_ps", bufs=2, space="PSUM") as hpsp, \
             tc.tile_pool(name="t_sb", bufs=10) as tp, \
             tc.tile_pool(name="s_sb", bufs=8) as sp, \
             tc.tile_pool(name="acc", bufs=2) as accp, \
             tc.tile_pool(name="out_ps", bufs=2, space="PSUM") as opsp, \
             tc.tile_pool(name="out_sb", bufs=2) as osbp:

            # x-path consts on the sync queue (first), other consts on the
            # scalar engine's queue so they never delay x blocks.
            rw_t = constp.tile([128, NV, C], f8)
            nc.sync.dma_start(rw_t[:], rw.ap().rearrange("p (vt c) -> p vt c", c=C))
            wwT_t = constp.tile([C, V], bf16)
            nc.scalar.dma_start(wwT_t[:], wwT.ap())
            opw_t = constp.tile([C, NOPS, C], bf16)
            nc.scalar.dma_start(opw_t[:], opw.ap().rearrange("p (i c) -> p i c", c=C))
            opb_t = constp.tile([C, NOPS], f32)
            nc.scalar.dma_start(opb_t[:], opb.ap())

            # read phase: x resident (fp8), both chunks' values accumulate
            # per block as x arrives; DoubleRow consumes v-tile pairs.
            xfull = xtp.tile([128, NV, T], f8)
            for blk in range(NBLK):
                nc.sync.dma_start(xfull[:, ts(blk, VB), :],
                                  xTr[:, ts(blk, VB), :])
            values = [vpsp.tile([128, TC], f32, name=f"values{i}")
                      for i in range(NTC)]
            for blk in range(NBLK):
                for j in range(VB // 2):
                    k = blk * (VB // 2) + j
                    for tcn in range(NTC):
                        for hh in range(2):
                            nc.tensor.matmul(
                                values[tcn][:, ts(hh, 512)],
                                rw_t[:, 2 * k:2 * k + 2, :],
                                xfull[:, 2 * k:2 * k + 2,
                                      tcn * TC + hh * 512:tcn * TC + (hh + 1) * 512],
                                start=(k == 0), stop=(k == NP - 1),
                                perf_mode=DR)

            for tcn in range(NTC):
                vals = vsbp.tile([128, TC], bf16)
                for hh in range(2):
                    nc.vector.tensor_copy(vals[:, ts(hh, 512)],
                                          values[tcn][:, ts(hh, 512)])

                # op bank: nonlins on ACT {1,2,5,6,7} and DVE {0,3,4},
                # mixture sum as a depth-3 tree on GpSimd.
                acc = accp.tile([128, TC], bf16)
                for hh in range(2):
                    tt = []
                    for i in range(NOPS):
                        h = hpsp.tile([128, 512], f32)
                        nc.tensor.matmul(h[:], opw_t[:, i, :],
                                         vals[:, ts(hh, 512)],
                                         start=True, stop=True)
                        t = tp.tile([128, 512], bf16)
                        if i == 0:
                            nc.vector.tensor_scalar(
                                t[:], h[:], pre[0], opb_t[:, 0:1],
                                op0=alu.mult, op1=alu.add)
                        elif i == 4:
                            nc.vector.tensor_scalar(
                                t[:], h[:], pre[4], opb_t[:, 4:5],
                                op0=alu.mult, op1=alu.add)
                        else:
                            fn = [None, AF.Relu, AF.Gelu, AF.Square, None,
                                  AF.Abs, AF.Tanh, AF.Sigmoid][i]
                            nc.scalar.activation(t[:], h[:], fn,
                                                 bias=opb_t[:, i:i + 1],
                                                 scale=pre[i])
                        tt.append(t)
                    s0 = sp.tile([128, 512], bf16)
                    nc.vector.scalar_tensor_tensor(s0[:], tt[2][:], post[2],
                                                   tt[0][:], op0=alu.mult,
                                                   op1=alu.add)
                    s1 = sp.tile([128, 512], bf16)
                    nc.vector.scalar_tensor_tensor(s1[:], tt[6][:], post[6],
                                                   tt[1][:], op0=alu.mult,
                                                   op1=alu.add)
                    s2 = sp.tile([128, 512], bf16)
                    nc.vector.scalar_tensor_tensor(s2[:], tt[7][:], post[7],
                                                   tt[3][:], op0=alu.mult,
                                                   op1=alu.add)
                    s3 = sp.tile([128, 512], bf16)
                    nc.gpsimd.tensor_tensor(s3[:], tt[4][:], tt[5][:],
                                            op=alu.add)
                    u0 = sp.tile([128, 512], bf16)
                    nc.gpsimd.tensor_tensor(u0[:], s0[:], s1[:], op=alu.add)
                    u1 = sp.tile([128, 512], bf16)
                    nc.gpsimd.tensor_tensor(u1[:], s2[:], s3[:], op=alu.add)
                    nc.gpsimd.tensor_tensor(acc[:, ts(hh, 512)], u0[:], u1[:],
                                            op=alu.add)

                # write: out rows; drains split DVE/ACT, stores on SWDGE
                for sub in range(TC // 128):
                    osb = osbp.tile([128, V], bf16)
                    for nn in range(8):
                        ops_ = opsp.tile([128, 512], f32)
                        nc.tensor.matmul(ops_[:], acc[:, ts(sub, 128)],
                                         wwT_t[:, ts(nn, 512)],
                                         start=True, stop=True)
                        if nn in (0, 3, 6):
                            nc.scalar.copy(osb[:, ts(nn, 512)], ops_[:])
                        else:
                            nc.vector.tensor_copy(osb[:, ts(nn, 512)], ops_[:])
                    row = tcn * (TC // 128) + sub
                    nc.gpsimd.dma_start(out.ap()[ts(row, 128), 0:2048],
                                        osb[:, 0:2048])
                    nc.gpsimd.dma_start(out.ap()[ts(row, 128), 2048:4096],
                                        osb[:, 2048:4096])
    nc.compile()
    return nc


def _softmax(x, axis):
    x = np.asarray(x, np.float32)
    m = x.max(axis=axis, keepdims=True)
    e = np.exp(x - m)
    return e / e.sum(axis=axis, keepdims=True)


def kernel(x, basis, read_coeffs, write_coeffs, op_logits, op_weights,
           op_biases, out_scale):
    global LAST_RESULT
    import ml_dtypes
    from concourse.bass_utils import run_bass_kernel_spmd
    bf16 = ml_dtypes.bfloat16
    f8 = ml_dtypes.float8_e4m3

    x = np.asarray(x, np.float32)
    basis = np.asarray(basis, np.float32)
    read_coeffs = np.asarray(read_coeffs, np.float32)
    write_coeffs = np.asarray(write_coeffs, np.float32)
    op_logits = np.asarray(op_logits, np.float32)
    op_weights = np.asarray(op_weights, np.float32)
    op_biases = np.asarray(op_biases, np.float32)
    out_scale = np.float32(out_scale)

    read_w = _softmax(basis @ read_coeffs.T, axis=0)               # (V, C)
    # pack read_w to [p, vt*C]; x256 so fp8 stays in normal range, the
    # dequant 1/256 is folded into the op-bank weights below
    rw_packed = np.ascontiguousarray(
        (read_w * RW_SCALE).reshape(NV, 128, C)
        .transpose(1, 0, 2).reshape(128, NV * C))
    wwT = np.ascontiguousarray((basis @ write_coeffs.T).T) * out_scale  # (C, V)
    w = _softmax(op_logits, axis=0).astype(np.float64)

    # fold the mixture weight into scale/bias where the nonlinearity allows
    #   i: 0 ident, 1 relu, 2 gelu, 3 square, 4 neg, 5 abs, 6 tanh, 7 sigmoid
    pre = [w[0], w[1], 1.0, np.sqrt(w[3]), -w[4], w[5], 1.0, 1.0]
    post = [1.0, 1.0, w[2], 1.0, 1.0, 1.0, w[6], w[7]]
    pre = [float(v) for v in pre]
    post = [float(v) for v in post]

    key = tuple(pre) + tuple(post)
    if key not in _CACHE:
        _CACHE[key] = _build(pre, post)
    nc = _CACHE[key]

    opb = (op_biases.T * np.array(pre, np.float64)[None, :]).astype(np.float32)
    # gelu/tanh/sigmoid biases enter before the nonlinearity unscaled
    for i in (2, 6, 7):
        opb[:, i] = op_biases[i]

    # opw packed to [p, i*C], with the read-path 1/RW_SCALE dequant folded in
    opw_packed = np.ascontiguousarray(
        (op_weights / RW_SCALE).transpose(1, 0, 2).reshape(C, NOPS * C))

    shared = {
        "rw": rw_packed.astype(f8),
        "wwT": wwT.astype(bf16),
        "opw": opw_packed.astype(bf16),
        "opb": np.ascontiguousarray(opb),
    }
    in_maps = []
    for b in range(B):
        m = dict(shared)
        m["xT"] = np.ascontiguousarray(x[b].T).astype(f8)
        in_maps.append(m)

    res = run_bass_kernel_spmd(nc, in_maps, core_ids=list(range(NCORES)))
    LAST_RESULT = res
    out = np.empty((B, T, V), np.float32)
    for b in range(B):
        out[b] = res.results[b]["out"].astype(np.float32)
    return out


# revision 14
# speedup vs baseline: 1.8047x; 1.1917x over previous
"""Trainium2 Bass kernel for the LGP-instruction module (read -> op bank -> write).

Data-parallel over batch: core b computes x[b] (2048, 4096).
Precision plan (gate 2e-2, measured ~3.3e-3): x and read_w in fp8-e4m3
(read_w scaled x256, dequant folded into op weights), everything else bf16.
Read phase uses DoubleRow fp8 matmuls (K=256 per instruction).

Per core: x stays SBUF-resident; both T-chunks' values accumulate block-by-
block as x streams in, then per chunk: op bank (ACT+DVE nonlins, GpSimd
mixture tree) and the write projection acc.T @ wwT with drains split
DVE/ACT and stores on SWDGE.
"""
import sys
import numpy as np

if '/opt/trn_rl_repo' not in sys.path:
    sys.path.insert(0, '/opt/trn_rl_repo')

B, T, V, C, NOPS = 8, 2048, 4096, 128, 8
NCORES = 8
NV = V // 128     # 32 v-tiles
NP = NV // 2      # 16 v-tile pairs (DoubleRow)
TC = 1024         # T-chunk
NTC = T // TC     # 2 T-chunks
RW_SCALE = 256.0

_CACHE = {}
LAST_RESULT = None


def _build(pre, post):
    from concourse import bass, bacc, tile, mybir
    f32, bf16, f8 = mybir.dt.float32, mybir.dt.bfloat16, mybir.dt.float8e4
    AF = mybir.ActivationFunctionType
    DR = mybir.MatmulPerfMode.DoubleRow
    ts = bass.ts
    alu = mybir.AluOpType

    nc = bacc.Bacc("TRN2", target_bir_lowering=False, debug=False,
                   num_devices=NCORES)
    xT = nc.dram_tensor("xT", [V, T], f8, kind="ExternalInput")
    rw = nc.dram_tensor("rw", [128, NV * C], f8, kind="ExternalInput")
    wwT = nc.dram_tensor("wwT", [C, V], bf16, kind="ExternalInput")
    opw = nc.dram_tensor("opw", [C, NOPS * C], bf16, kind="ExternalInput")
    opb = nc.dram_tensor("opb", [C, NOPS], f32, kind="ExternalInput")
    out = nc.dram_tensor("out", [T, V], bf16, kind="ExternalOutput")

    NBLK = 4          # xT load blocks (full T each)
    VB = NV // NBLK   # 8 v-tiles per block

    xTr = xT.ap().rearrange("(vb p) t -> p vb t", p=128)

    with tile.TileContext(nc) as tc:
        with tc.tile_pool(name="const", bufs=1) as constp, \
             tc.tile_pool(name="xt", bufs=1) as xtp, \
             tc.tile_pool(name="vals_ps", bufs=1, space="PSUM") as vpsp, \
             tc.tile_pool(name="vals_sb", bufs=2) as vsbp, \
             tc.tile_pool(name="hw_ps", bufs=4, space="PSUM") as hwpsp, \
             tc.tile_pool(name="t_sb", bufs=10) as tp, \
             tc.tile_pool(name="s_sb", bufs=8) as sp, \
             tc.tile_pool(name="acc", bufs=2) as accp, \
             tc.tile_pool(name="out_sb", bufs=3) as osbp:

            # x-path consts on the sync queue (first), other consts on the
            # scalar engine's queue so they never delay x blocks.
            rw_t = constp.tile([128, NV, C], f8)
            nc.sync.dma_start(rw_t[:], rw.ap().rearrange("p (vt c) -> p vt c", c=C))
            wwT_t = constp.tile([C, V], bf16)
            nc.scalar.dma_start(wwT_t[:], wwT.ap())
            opw_t = constp.tile([C, NOPS, C], bf16)
            nc.scalar.dma_start(opw_t[:], opw.ap().rearrange("p (i c) -> p i c", c=C))
            opb_t = constp.tile([C, NOPS], f32)
            nc.scalar.dma_start(opb_t[:], opb.ap())

            # read phase: x resident (fp8), both chunks' values accumulate
            # per block as x arrives; DoubleRow consumes v-tile pairs.
            xfull = xtp.tile([128, NV, T], f8)
            for blk in range(NBLK):
                nc.sync.dma_start(xfull[:, ts(blk, VB), :],
                                  xTr[:, ts(blk, VB), :])
            values = [vpsp.tile([128, TC], f32, name=f"values{i}")
                      for i in range(NTC)]
            for blk in range(NBLK):
                for j in range(VB // 2):
                    k = blk * (VB // 2) + j
                    for tcn in range(NTC):
                        for hh in range(2):
                            nc.tensor.matmul(
                                values[tcn][:, ts(hh, 512)],
                                rw_t[:, 2 * k:2 * k + 2, :],
                                xfull[:, 2 * k:2 * k + 2,
                                      tcn * TC + hh * 512:tcn * TC + (hh + 1) * 512],
                                start=(k == 0), stop=(k == NP - 1),
                                perf_mode=DR)

            for tcn in range(NTC):
                vals = vsbp.tile([128, TC], bf16)
                for hh in range(2):
                    nc.vector.tensor_copy(vals[:, ts(hh, 512)],
                                          values[tcn][:, ts(hh, 512)])

                # op bank: nonlins on ACT {1,2,5,6,7} and DVE {0,3,4},
                # mixture sum as a depth-3 tree on GpSimd.
                acc = accp.tile([128, TC], bf16)
                for hh in range(2):
                    tt = []
                    for i in range(NOPS):
                        h = hwpsp.tile([128, 512], f32, name="hw")
                        nc.tensor.matmul(h[:], opw_t[:, i, :],
                                         vals[:, ts(hh, 512)],
                                         start=True, stop=True)
                        t = tp.tile([128, 512], bf16)
                        if i == 0:
                            nc.vector.tensor_scalar(
                                t[:], h[:], pre[0], opb_t[:, 0:1],
                                op0=alu.mult, op1=alu.add)
                        elif i == 4:
                            nc.vector.tensor_scalar(
                                t[:], h[:], pre[4], opb_t[:, 4:5],
                                op0=alu.mult, op1=alu.add)
                        else:
                            fn = [None, AF.Relu, AF.Gelu, AF.Square, None,
                                  AF.Abs, AF.Tanh, AF.Sigmoid][i]
                            nc.scalar.activation(t[:], h[:], fn,
                                                 bias=opb_t[:, i:i + 1],
                                                 scale=pre[i])
                        tt.append(t)
                    s0 = sp.tile([128, 512], bf16)
                    nc.vector.scalar_tensor_tensor(s0[:], tt[2][:], post[2],
                                                   tt[0][:], op0=alu.mult,
                                                   op1=alu.add)
                    s1 = sp.tile([128, 512], bf16)
                    nc.vector.scalar_tensor_tensor(s1[:], tt[6][:], post[6],
                                                   tt[1][:], op0=alu.mult,
                                                   op1=alu.add)
                    s2 = sp.tile([128, 512], bf16)
                    nc.vector.scalar_tensor_tensor(s2[:], tt[7][:], post[7],
                                                   tt[3][:], op0=alu.mult,
                                                   op1=alu.add)
                    s3 = sp.tile([128, 512], bf16)
                    nc.gpsimd.tensor_tensor(s3[:], tt[4][:], tt[5][:],
                                            op=alu.add)
                    u0 = sp.tile([128, 512], bf16)
                    nc.gpsimd.tensor_tensor(u0[:], s0[:], s1[:], op=alu.add)
                    u1 = sp.tile([128, 512], bf16)
                    nc.gpsimd.tensor_tensor(u1[:], s2[:], s3[:], op=alu.add)
                    nc.gpsimd.tensor_tensor(acc[:, ts(hh, 512)], u0[:], u1[:],
                                            op=alu.add)

                # write: out rows; drains split DVE/ACT, stores on SWDGE
                for sub in range(TC // 128):
                    osb = osbp.tile([128, V], bf16)
                    for nn in range(8):
                        ops_ = hwpsp.tile([128, 512], f32, name="hw")
                        nc.tensor.matmul(ops_[:], acc[:, ts(sub, 128)],
                                         wwT_t[:, ts(nn, 512)],
                                         start=True, stop=True)
                        if nn in (0, 3, 6):
                            nc.scalar.copy(osb[:, ts(nn, 512)], ops_[:])
                        else:
                            nc.vector.tensor_copy(osb[:, ts(nn, 512)], ops_[:])
                    row = tcn * (TC // 128) + sub
                    nc.gpsimd.dma_start(out.ap()[ts(row, 128), 0:2048],
                                        osb[:, 0:2048])
                    nc.gpsimd.dma_start(out.ap()[ts(row, 128), 2048:4096],
                                        osb[:, 2048:4096])
    nc.compile()
    return nc


def _softmax(x, axis):
    x = np.asarray(x, np.float32)
    m = x.max(axis=axis, keepdims=True)
    e = np.exp(x - m)
    return e / e.sum(axis=axis, keepdims=True)


def kernel(x, basis, read_coeffs, write_coeffs, op_logits, op_weights,
           op_biases, out_scale):
    global LAST_RESULT
    import ml_dtypes
    from concourse.bass_utils import run_bass_kernel_spmd
    bf16 = ml_dtypes.bfloat16
    f8 = ml_dtypes.float8_e4m3

    x = np.asarray(x, np.float32)
    basis = np.asarray(basis, np.float32)
    read_coeffs = np.asarray(read_coeffs, np.float32)
    write_coeffs = np.asarray(write_coeffs, np.float32)
    op_logits = np.asarray(op_logits, np.float32)
    op_weights = np.asarray(op_weights, np.float32)
    op_biases = np.asarray(op_biases, np.float32)
    out_scale = np.float32(out_scale)

    read_w = _softmax(basis @ read_coeffs.T, axis=0)               # (V, C)
    # pack read_w to [p, vt*C]; x256 so fp8 stays in normal range, the
    # dequant 1/256 is folded into the op-bank weights below
    rw_packed = np.ascontiguousarray(
        (read_w * RW_SCALE).reshape(NV, 128, C)
        .transpose(1, 0, 2).reshape(128, NV * C))
    wwT = np.ascontiguousarray((basis @ write_coeffs.T).T) * out_scale  # (C, V)
    w = _softmax(op_logits, axis=0).astype(np.float64)

    # fold the mixture weight into scale/bias where the nonlinearity allows
    #   i: 0 ident, 1 relu, 2 gelu, 3 square, 4 neg, 5 abs, 6 tanh, 7 sigmoid
    pre = [w[0], w[1], 1.0, np.sqrt(w[3]), -w[4], w[5], 1.0, 1.0]
    post = [1.0, 1.0, w[2], 1.0, 1.0, 1.0, w[6], w[7]]
    pre = [float(v) for v in pre]
    post = [float(v) for v in post]

    key = tuple(pre) + tuple(post)
    if key not in _CACHE:
        _CACHE[key] = _build(pre, post)
    nc = _CACHE[key]

    opb = (op_biases.T * np.array(pre, np.float64)[None, :]).astype(np.float32)
    # gelu/tanh/sigmoid biases enter before the nonlinearity unscaled
    for i in (2, 6, 7):
        opb[:, i] = op_biases[i]

    # opw packed to [p, i*C], with the read-path 1/RW_SCALE dequant folded in
    opw_packed = np.ascontiguousarray(
        (op_weights / RW_SCALE).transpose(1, 0, 2).reshape(C, NOPS * C))

    shared = {
        "rw": rw_packed.astype(f8),
        "wwT": wwT.astype(bf16),
        "opw": opw_packed.astype(bf16),
        "opb": np.ascontiguousarray(opb),
    }
    in_maps = []
    for b in range(B):
        m = dict(shared)
        m["xT"] = np.ascontiguousarray(x[b].T).astype(f8)
        in_maps.append(m)

    res = run_bass_kernel_spmd(nc, in_maps, core_ids=list(range(NCORES)))
    LAST_RESULT = res
    out = np.empty((B, T, V), np.float32)
    for b in range(B):
        out[b] = res.results[b]["out"].astype(np.float32)
    return out


# revision 15
# speedup vs baseline: 1.9286x; 1.0687x over previous
"""Trainium2 Bass kernel for the LGP-instruction module (read -> op bank -> write).

Data-parallel over batch: core b computes x[b] (2048, 4096).
Precision plan (gate 2e-2, measured ~3.9e-3): x and read_w in fp8-e4m3
(read_w scaled x256, dequant folded into op weights), everything else bf16.
Read phase uses DoubleRow fp8 matmuls (K=256 per instruction).

Pipeline per core, 4 T-chunks of 512:
  x loads in two T-half blocks (4 sub-DMAs each); chunks 0/1 accumulate as
  block 0 streams in, chunk 0's op-bank/write/store phase starts right after,
  with block 1's read matmuls interleaved between chunk-0 write bursts as
  PE filler while drains catch up.
  op bank: ACT {relu,square,abs,gelu,tanh,tanh(=sigmoid)} + DVE {ident,neg},
  mixture summed by a short tree (DVE + GpSimd).
  write: acc.T @ wwT (bf16), drains alternate DVE/ACT, stores alternate
  SWDGE (gpsimd) / HWDGE (sync).
"""
import sys
import numpy as np

if '/opt/trn_rl_repo' not in sys.path:
    sys.path.insert(0, '/opt/trn_rl_repo')

B, T, V, C, NOPS = 8, 2048, 4096, 128, 8
NCORES = 8
NV = V // 128     # 32 v-tiles
NP = NV // 2      # 16 v-tile pairs (DoubleRow)
TC = 512          # T-chunk
NTC = T // TC     # 4 T-chunks
RW_SCALE = 256.0

_CACHE = {}
LAST_RESULT = None


def _build(pre, post):
    from concourse import bass, bacc, tile, mybir
    f32, bf16, f8 = mybir.dt.float32, mybir.dt.bfloat16, mybir.dt.float8e4
    AF = mybir.ActivationFunctionType
    DR = mybir.MatmulPerfMode.DoubleRow
    ts = bass.ts
    alu = mybir.AluOpType

    nc = bacc.Bacc("TRN2", target_bir_lowering=False, debug=False,
                   num_devices=NCORES)
    xT = nc.dram_tensor("xT", [V, T], f8, kind="ExternalInput")
    rw = nc.dram_tensor("rw", [128, NV * C], f8, kind="ExternalInput")
    wwT = nc.dram_tensor("wwT", [C, V], bf16, kind="ExternalInput")
    opw = nc.dram_tensor("opw", [C, NOPS * C], bf16, kind="ExternalInput")
    opb = nc.dram_tensor("opb", [C, NOPS], f32, kind="ExternalInput")
    out = nc.dram_tensor("out", [T, V], bf16, kind="ExternalOutput")

    xTr = xT.ap().rearrange("(vb p) t -> p vb t", p=128)

    with tile.TileContext(nc) as tc:
        with tc.tile_pool(name="const", bufs=1) as constp, \
             tc.tile_pool(name="xt", bufs=1) as xtp, \
             tc.tile_pool(name="vals_ps", bufs=1, space="PSUM") as vpsp, \
             tc.tile_pool(name="vals_sb", bufs=1) as vsbp, \
             tc.tile_pool(name="hw_ps", bufs=4, space="PSUM") as hwpsp, \
             tc.tile_pool(name="t_sb", bufs=10) as tp, \
             tc.tile_pool(name="s_sb", bufs=8) as sp, \
             tc.tile_pool(name="acc", bufs=2) as accp, \
             tc.tile_pool(name="out_sb", bufs=3) as osbp:

            # x-path consts first on the sync queue; the rest on the scalar
            # engine's queue so they never delay x blocks.
            rw_t = constp.tile([128, NV, C], f8)
            nc.sync.dma_start(rw_t[:], rw.ap().rearrange("p (vt c) -> p vt c", c=C))
            wwT_t = constp.tile([C, V], bf16)
            nc.scalar.dma_start(wwT_t[:], wwT.ap())
            opw_t = constp.tile([C, NOPS, C], bf16)
            nc.scalar.dma_start(opw_t[:], opw.ap().rearrange("p (i c) -> p i c", c=C))
            opb_t = constp.tile([C, NOPS], f32)
            nc.scalar.dma_start(opb_t[:], opb.ap())

            # x resident in fp8; two T-half blocks of 4 sub-DMAs each
            xfull = xtp.tile([128, NV, T], f8)
            for bt in range(2):
                for vb in range(4):
                    nc.sync.dma_start(
                        xfull[:, ts(vb, 8), ts(bt, 1024)],
                        xTr[:, ts(vb, 8), ts(bt, 1024)])

            values = [vpsp.tile([128, TC], f32, name=f"values{i}")
                      for i in range(NTC)]

            def read_mms(bt):
                # 32 DoubleRow matmuls covering chunks (2*bt, 2*bt+1)
                g = []
                for vb in range(4):
                    for j in range(4):
                        k = vb * 4 + j
                        for lc in range(2):
                            c = bt * 2 + lc
                            g.append((k, c))
                return g

            def emit_read(k, c):
                nc.tensor.matmul(
                    values[c][:], rw_t[:, 2 * k:2 * k + 2, :],
                    xfull[:, 2 * k:2 * k + 2, ts(c, TC)],
                    start=(k == 0), stop=(k == NP - 1), perf_mode=DR)

            for k, c in read_mms(0):
                emit_read(k, c)

            vals = {}

            def emit_vals_copy(c):
                v = vsbp.tile([128, TC], bf16, name=f"vals{c}")
                nc.vector.tensor_copy(v[:], values[c][:])
                vals[c] = v

            emit_vals_copy(0)
            emit_vals_copy(1)

            filler0 = read_mms(1)  # interleaved into chunk 0's write phase

            def process_chunk(c, filler):
                # op bank; ACT-assigned ops first so ACT streams early
                acc = accp.tile([128, TC], bf16)
                tt = {}
                for i in (1, 3, 5, 2, 6, 7, 0, 4):
                    h = hwpsp.tile([128, TC], f32, name="hw")
                    nc.tensor.matmul(h[:], opw_t[:, i, :], vals[c][:],
                                     start=True, stop=True)
                    t = tp.tile([128, TC], bf16)
                    if i == 0:
                        nc.vector.tensor_scalar(
                            t[:], h[:], pre[0], opb_t[:, 0:1],
                            op0=alu.mult, op1=alu.add)
                    elif i == 4:
                        nc.vector.tensor_scalar(
                            t[:], h[:], pre[4], opb_t[:, 4:5],
                            op0=alu.mult, op1=alu.add)
                    else:
                        fn = [None, AF.Relu, AF.Gelu, AF.Square, None,
                              AF.Abs, AF.Tanh, AF.Tanh][i]
                        nc.scalar.activation(t[:], h[:], fn,
                                             bias=opb_t[:, i:i + 1],
                                             scale=pre[i])
                    tt[i] = t
                # mixture tree: depth 3; heavy path on DVE, side adds on Pool
                s3 = sp.tile([128, TC], bf16)
                nc.gpsimd.tensor_tensor(s3[:], tt[4][:], tt[5][:], op=alu.add)
                s0 = sp.tile([128, TC], bf16)
                nc.vector.scalar_tensor_tensor(s0[:], tt[2][:], post[2],
                                               tt[0][:], op0=alu.mult,
                                               op1=alu.add)
                s1 = sp.tile([128, TC], bf16)
                nc.vector.scalar_tensor_tensor(s1[:], tt[6][:], post[6],
                                               tt[1][:], op0=alu.mult,
                                               op1=alu.add)
                u0 = sp.tile([128, TC], bf16)
                nc.gpsimd.tensor_tensor(u0[:], s0[:], s1[:], op=alu.add)
                s2 = sp.tile([128, TC], bf16)
                nc.vector.scalar_tensor_tensor(s2[:], tt[7][:], post[7],
                                               tt[3][:], op0=alu.mult,
                                               op1=alu.add)
                u1 = sp.tile([128, TC], bf16)
                nc.vector.tensor_tensor(u1[:], s2[:], s3[:], op=alu.add)
                nc.vector.tensor_tensor(acc[:], u0[:], u1[:], op=alu.add)

                # write phase
                for sub in range(TC // 128):
                    osb = osbp.tile([128, V], bf16)
                    for nn in range(8):
                        ops_ = hwpsp.tile([128, TC], f32, name="hw")
                        nc.tensor.matmul(ops_[:], acc[:, ts(sub, 128)],
                                         wwT_t[:, ts(nn, 512)],
                                         start=True, stop=True)
                        if (nn + sub) % 2 == 0:
                            nc.scalar.copy(osb[:, ts(nn, 512)], ops_[:])
                        else:
                            nc.vector.tensor_copy(osb[:, ts(nn, 512)], ops_[:])
                    row = c * (TC // 128) + sub
                    nc.gpsimd.dma_start(out.ap()[ts(row, 128), 0:2048],
                                        osb[:, 0:2048])
                    nc.sync.dma_start(out.ap()[ts(row, 128), 2048:4096],
                                      osb[:, 2048:4096])
                    # PE filler while drains catch up: block-1 read matmuls
                    for k, cc in filler[sub * 8:(sub + 1) * 8]:
                        emit_read(k, cc)

            process_chunk(0, filler0)
            emit_vals_copy(2)
            emit_vals_copy(3)
            process_chunk(1, [])
            process_chunk(2, [])
            process_chunk(3, [])
    nc.compile()
    return nc


def _softmax(x, axis):
    x = np.asarray(x, np.float32)
    m = x.max(axis=axis, keepdims=True)
    e = np.exp(x - m)
    return e / e.sum(axis=axis, keepdims=True)


def kernel(x, basis, read_coeffs, write_coeffs, op_logits, op_weights,
           op_biases, out_scale):
    global LAST_RESULT
    import ml_dtypes
    from concourse.bass_utils import run_bass_kernel_spmd
    bf16 = ml_dtypes.bfloat16
    f8 = ml_dtypes.float8_e4m3

    x = np.asarray(x, np.float32)
    basis = np.asarray(basis, np.float32)
    read_coeffs = np.asarray(read_coeffs, np.float32)
    write_coeffs = np.asarray(write_coeffs, np.float32)
    op_logits = np.asarray(op_logits, np.float32)
    op_weights = np.asarray(op_weights, np.float32)
    op_biases = np.asarray(op_biases, np.float32)
    out_scale = np.float32(out_scale)

    read_w = _softmax(basis @ read_coeffs.T, axis=0)               # (V, C)
    # pack read_w to [p, vt*C]; x256 so fp8 stays in normal range, the
    # dequant 1/256 is folded into the op-bank weights below
    rw_packed = np.ascontiguousarray(
        (read_w * RW_SCALE).reshape(NV, 128, C)
        .transpose(1, 0, 2).reshape(128, NV * C))
    wwT = np.ascontiguousarray((basis @ write_coeffs.T).T) * out_scale  # (C, V)
    w = _softmax(op_logits, axis=0).astype(np.float64)

    # fold the mixture weight into scale/bias where the nonlinearity allows
    #   i: 0 ident, 1 relu, 2 gelu, 3 square, 4 neg, 5 abs, 6 tanh, 7 sigmoid
    # sigmoid(z) = 1/2 + tanh(z/2)/2: runs as Tanh with scale 0.5 and
    # post w7/2; the constant w7/2 is folded into the identity op's bias.
    pre = [w[0], w[1], 1.0, np.sqrt(w[3]), -w[4], w[5], 1.0, 0.5]
    post = [1.0, 1.0, w[2], 1.0, 1.0, 1.0, w[6], w[7] / 2.0]
    pre = [float(v) for v in pre]
    post = [float(v) for v in post]

    key = tuple(pre) + tuple(post)
    if key not in _CACHE:
        _CACHE[key] = _build(pre, post)
    nc = _CACHE[key]

    opb = (op_biases.T * np.array(pre, np.float64)[None, :]).astype(np.float32)
    # gelu/tanh biases enter before the nonlinearity unscaled; the
    # sigmoid-as-tanh op needs bias b7/2; identity carries the w7/2 constant
    opb[:, 2] = op_biases[2]
    opb[:, 6] = op_biases[6]
    opb[:, 7] = 0.5 * op_biases[7]
    opb[:, 0] += float(w[7]) / 2.0

    # opw packed to [p, i*C], with the read-path 1/RW_SCALE dequant folded in
    opw_packed = np.ascontiguousarray(
        (op_weights / RW_SCALE).transpose(1, 0, 2).reshape(C, NOPS * C))

    shared = {
        "rw": rw_packed.astype(f8),
        "wwT": wwT.astype(bf16),
        "opw": opw_packed.astype(bf16),
        "opb": np.ascontiguousarray(opb),
    }
    in_maps = []
    for b in range(B):
        m = dict(shared)
        m["xT"] = np.ascontiguousarray(x[b].T).astype(f8)
        in_maps.append(m)

    res = run_bass_kernel_spmd(nc, in_maps, core_ids=list(range(NCORES)))
    LAST_RESULT = res
    out = np.empty((B, T, V), np.float32)
    for b in range(B):
        out[b] = res.results[b]["out"].astype(np.float32)
    return out
